# revision 10
# baseline (speedup 1.0000x reference)
# Multi-head attention kernel for Trainium2, sharded over 8 NeuronCores.
#
# Sharding: core = (batch b, query-chunk qc). Each core handles QB=512 queries
# of one batch, all 12 heads, recomputing the K/V projections for its batch
# (cheaper than cross-core collectives on this chip).
#
# Layout strategy (all fp32):
#   - Host pre-transposes activations to [E, S] so the contraction dim (E)
#     lands on SBUF partitions.
#   - q^T, k^T computed as [768, S] via lhsT=W chunks; per-partition bias
#     added during PSUM->SBUF copy.
#   - v computed directly as [keys, 768] using x_v^T chunks as the stationary
#     operand; stored with a ones-column per head ([128,16,12,65]) so the PV
#     matmul (M=65) also produces the softmax denominator row.
#   - scores^T = [keys, queries] per head: K=64 matmuls; even/odd heads sit in
#     partition halves 0-63/64-127 so adjacent matmuls land in disjoint PE row
#     groups and can run concurrently.
#   - exp on ScalarE in [128, 2*512] groups, PSUM->SBUF, streamed straight
#     into the accumulating PV matmul (no full score matrix ever lives in SBUF).
#   - normalize: reciprocal of denom row + gpsimd partition_broadcast + DVE mul.
#   - output projection contracts d_all via K=64 chunks (head-pair row packed),
#     epilogue adds host-precomputed bias (bv folded through Wo).

import numpy as np
from contextlib import ExitStack

import concourse.bass as bass
import concourse.mybir as mybir
import concourse.tile as tile
from concourse import bacc
from concourse.bass_utils import run_bass_kernel_spmd

F32 = mybir.dt.float32
P = 128
E = 768
S = 2048
B = 2
H = 12
D = 64
QB = 512          # queries per core
NCORES = 8
EC = E // P       # 6 e-chunks
KT = S // P       # 16 key tiles
MT_Q = E // P     # 6 M-tiles for q^T/k^T (768 rows)
NC4 = S // 512    # 4 n-slices of k^T


def build_nc():
    nc = bacc.Bacc("TRN2", debug=False)

    # DRAM I/O (per-core shapes; same NEFF on all 8 cores)
    xq = nc.dram_tensor("xq", (E, QB), F32, kind="ExternalInput")     # query[b,chunk].T
    xk = nc.dram_tensor("xk", (E, S), F32, kind="ExternalInput")      # key[b].T
    xv = nc.dram_tensor("xv", (E, S), F32, kind="ExternalInput")      # value[b].T
    wq = nc.dram_tensor("wq", (E, E), F32, kind="ExternalInput")      # [E, H*D], pre-scaled 1/sqrt(D)
    wk = nc.dram_tensor("wk", (E, E), F32, kind="ExternalInput")
    wv = nc.dram_tensor("wv", (E, E), F32, kind="ExternalInput")
    wo = nc.dram_tensor("wo", (E, E), F32, kind="ExternalInput")
    bq = nc.dram_tensor("bq", (P, MT_Q), F32, kind="ExternalInput")   # per-partition bias per M-tile
    bk = nc.dram_tensor("bk", (P, MT_Q), F32, kind="ExternalInput")
    bo = nc.dram_tensor("bo", (P, E), F32, kind="ExternalInput")      # bv@Wo + bo, broadcast
    out = nc.dram_tensor("out", (QB, E), F32, kind="ExternalOutput")

    with tile.TileContext(nc) as tc:
        with ExitStack() as ctx:
            _emit(ctx, tc, nc, xq, xk, xv, wq, wk, wv, wo, bq, bk, bo, out)
    nc.compile()
    return nc


def _emit(ctx, tc, nc, xq, xk, xv, wq, wk, wv, wo, bq, bk, bo, out):
    # ---- pools ----
    # SBUF persistent
    persist = ctx.enter_context(tc.tile_pool(name="persist", bufs=1))
    # big weight slots [128, 6, 768] reused wq -> wk -> wv -> wo
    wpool = ctx.enter_context(tc.tile_pool(name="wpool", bufs=2))
    # x input slices
    xpool = ctx.enter_context(tc.tile_pool(name="xpool", bufs=2))
    xvpool = ctx.enter_context(tc.tile_pool(name="xvpool", bufs=2))
    # exp output stream
    epool = ctx.enter_context(tc.tile_pool(name="epool", bufs=2))
    # small temps
    spool = ctx.enter_context(tc.tile_pool(name="spool", bufs=1))
    outpool = ctx.enter_context(tc.tile_pool(name="outpool", bufs=1))
    # PSUM pools
    psA = ctx.enter_context(tc.tile_pool(name="psA", bufs=2, space="PSUM"))   # [128,512] proj qk + PV out
    psB = ctx.enter_context(tc.tile_pool(name="psB", bufs=1, space="PSUM"))   # [128,768] v proj + out proj
    psC = ctx.enter_context(tc.tile_pool(name="psC", bufs=2, space="PSUM"))   # [128,2,512] scores

    # ---- persistent SBUF tensors ----
    qT = persist.tile([P, MT_Q, QB], F32)       # q^T [768, QB]
    kT = persist.tile([P, MT_Q, S], F32)        # k^T [768, S]
    v_sb = persist.tile([P, KT, H, D + 1], F32)  # v + ones column per head
    o_all = persist.tile([P, H // 2, QB], F32)   # normalized o^T, head pairs in partition halves
    bq_sb = persist.tile([P, MT_Q], F32)
    bk_sb = persist.tile([P, MT_Q], F32)
    bo_sb = persist.tile([P, E], F32)
    ones_sb = persist.tile([D + 1, D], F32)  # row D holds ones, base partition D

    nc.sync.dma_start(bq_sb[:], bq[:])
    nc.sync.dma_start(bk_sb[:], bk[:])
    nc.sync.dma_start(bo_sb[:], bo[:])

    # ones columns for denominator (written once; v-proj copies don't touch col D)
    nc.vector.memset(v_sb[:, :, :, D], 1.0)
    nc.vector.memset(ones_sb[D:D + 1, :], 1.0)

    wq_t = wpool.tile([P, EC, E], F32, tag="w18")
    nc.sync.dma_start(wq_t[:], wq[:].rearrange("(ec p) m -> p ec m", p=P))

    # ---- q^T projection ----
    xq_t = xpool.tile([P, EC, QB], F32, tag="xs")
    nc.sync.dma_start(xq_t[:], xq[:].rearrange("(ec p) s -> p ec s", p=P))
    for mt in range(MT_Q):
        ps = psA.tile([P, 512], F32, tag="psA")
        for ec in range(EC):
            nc.tensor.matmul(ps[:], wq_t[:, ec, mt * P:(mt + 1) * P], xq_t[:, ec, :],
                             start=(ec == 0), stop=(ec == EC - 1))
        nc.vector.tensor_scalar_add(qT[:, mt, :], ps[:], bq_sb[:, mt:mt + 1])

    # ---- k^T projection ----
    wk_t = wpool.tile([P, EC, E], F32, tag="w18")
    nc.sync.dma_start(wk_t[:], wk[:].rearrange("(ec p) m -> p ec m", p=P))
    for n4 in range(NC4):
        xk_t = xpool.tile([P, EC, 512], F32, tag="xs")
        nc.sync.dma_start(xk_t[:], xk[:, n4 * 512:(n4 + 1) * 512].rearrange("(ec p) s -> p ec s", p=P))
        for mt in range(MT_Q):
            ps = psA.tile([P, 512], F32, tag="psA")
            for ec in range(EC):
                nc.tensor.matmul(ps[:], wk_t[:, ec, mt * P:(mt + 1) * P], xk_t[:, ec, :],
                                 start=(ec == 0), stop=(ec == EC - 1))
            nc.vector.tensor_scalar_add(kT[:, mt, n4 * 512:(n4 + 1) * 512], ps[:], bk_sb[:, mt:mt + 1])

    # ---- v projection (direct [keys, d]; no bias — folded into bo host-side) ----
    wv_t = wpool.tile([P, EC, E], F32, tag="w18")
    nc.sync.dma_start(wv_t[:], wv[:].rearrange("(ec p) m -> p ec m", p=P))
    for kt in range(KT):
        xv_t = xvpool.tile([P, EC, P], F32, tag="xv")
        nc.sync.dma_start(xv_t[:], xv[:, kt * P:(kt + 1) * P].rearrange("(ec p) s -> p ec s", p=P))
        psv = psB.tile([P, E], F32, tag="psB")
        for ec in range(EC):
            nc.tensor.matmul(psv[:, 0:512], xv_t[:, ec, :], wv_t[:, ec, 0:512],
                             start=(ec == 0), stop=(ec == EC - 1))
            nc.tensor.matmul(psv[:, 512:768], xv_t[:, ec, :], wv_t[:, ec, 512:768],
                             start=(ec == 0), stop=(ec == EC - 1))
        # strided copy into per-head slots (leaves ones column intact)
        nc.vector.tensor_copy(v_sb[:, kt, :, 0:D], psv[:].rearrange("p (h d) -> p h d", d=D))

    # ---- attention: head pairs ----
    NG = KT // 2  # 8 groups of 2 key tiles
    for hp in range(H // 2):
        o_ps = {}
        for i in range(2):
            o_ps[i] = psA.tile([P, 512], F32, tag="psA", name=f"o_ps{i}")
        for g in range(NG):
            for i in range(2):
                h = 2 * hp + i
                po = D * i      # partition offset of this head's d-rows
                mt = h // 2
                st = psC.tile([P, 2, 512], F32, tag="psC")
                for j in range(2):
                    kt = 2 * g + j
                    nc.tensor.matmul(st[:, j, :],
                                     kT[po:po + D, mt, kt * P:(kt + 1) * P],
                                     qT[po:po + D, mt, :],
                                     start=True, stop=True)
                ex = epool.tile([P, 2, 512], F32, tag="ex")
                nc.scalar.activation(ex[:, :, :], st[:, :, :], mybir.ActivationFunctionType.Exp)
                for j in range(2):
                    kt = 2 * g + j
                    nc.tensor.matmul(o_ps[i][0:D + 1, :],
                                     v_sb[:, kt, h, :],
                                     ex[:, j, :],
                                     start=(g == 0 and j == 0), stop=(g == NG - 1 and j == 1))
        # normalize: recip of denom row, PE-broadcast to 64 partitions, multiply
        for i in range(2):
            po = D * i
            rec = spool.tile([D + 1, 512], F32, tag="rec", name=f"rec{i}")
            nc.vector.reciprocal(rec[D:D + 1, :], o_ps[i][D:D + 1, :])
            bc_ps = psB.tile([P, E], F32, tag="psB", name=f"bc{i}")
            nc.tensor.matmul(bc_ps[0:D, 0:512], ones_sb[D:D + 1, :], rec[D:D + 1, :],
                             start=True, stop=True)
            bc_sb = spool.tile([D, 512], F32, tag="rb", name=f"bc_sb{i}")
            nc.scalar.copy(bc_sb[:], bc_ps[0:D, 0:512])
            nc.vector.tensor_tensor(o_all[po:po + D, hp, :], o_ps[i][0:D, :], bc_sb[:],
                                    mybir.AluOpType.mult)

    # ---- output projection ----
    wo_t = wpool.tile([P, EC, E], F32, tag="w18")
    nc.sync.dma_start(wo_t[:], wo[:].rearrange("(ec p) m -> p ec m", p=P))
    ST = QB // P  # 4 s-tiles
    for st4 in range(ST):
        op = psB.tile([P, E], F32, tag="psB")
        for hp in range(H // 2):
            # both heads of the pair contract in one K=128 matmul
            first = (hp == 0)
            last = (hp == H // 2 - 1)
            nc.tensor.matmul(op[:, 0:512],
                             o_all[:, hp, st4 * P:(st4 + 1) * P],
                             wo_t[:, hp, 0:512],
                             start=first, stop=last)
            nc.tensor.matmul(op[:, 512:768],
                             o_all[:, hp, st4 * P:(st4 + 1) * P],
                             wo_t[:, hp, 512:768],
                             start=first, stop=last)
        out_sb = outpool.tile([P, E], F32, tag="outsb")
        nc.vector.tensor_tensor(out_sb[:], op[:], bo_sb[:], mybir.AluOpType.add)
        nc.sync.dma_start(out[st4 * P:(st4 + 1) * P, :], out_sb[:])


_NC_CACHE = None


def _get_nc():
    global _NC_CACHE
    if _NC_CACHE is None:
        _NC_CACHE = build_nc()
    return _NC_CACHE


def make_in_maps(query, key_, value, Wq, bq, Wk, bk, Wv, bv, Wo, bo):
    """Host-side sharding + layout prep. Returns list of 8 input dicts."""
    query = np.asarray(query, dtype=np.float32)
    key_ = np.asarray(key_, dtype=np.float32)
    value = np.asarray(value, dtype=np.float32)
    scale = 1.0 / np.sqrt(np.float32(D))

    wq_f = (np.ascontiguousarray(np.transpose(np.asarray(Wq, np.float32), (1, 0, 2)).reshape(E, E)) * scale)
    wk_f = np.ascontiguousarray(np.transpose(np.asarray(Wk, np.float32), (1, 0, 2)).reshape(E, E))
    wv_f = np.ascontiguousarray(np.transpose(np.asarray(Wv, np.float32), (1, 0, 2)).reshape(E, E))
    wo_f = np.ascontiguousarray(np.asarray(Wo, np.float32))

    bq_f = (np.asarray(bq, np.float32).reshape(E) * scale).reshape(MT_Q, P).T.copy()
    bk_f = np.asarray(bk, np.float32).reshape(E).reshape(MT_Q, P).T.copy()
    bv_f = np.asarray(bv, np.float32).reshape(E)
    bo_eff = np.tile((bv_f @ wo_f + np.asarray(bo, np.float32)).reshape(1, E), (P, 1)).copy()

    xk_t = [np.ascontiguousarray(key_[b].T) for b in range(B)]
    xv_t = [np.ascontiguousarray(value[b].T) for b in range(B)]

    in_maps = []
    for core in range(NCORES):
        b = core // (NCORES // B)
        qc = core % (NCORES // B)
        xq_t = np.ascontiguousarray(query[b, qc * QB:(qc + 1) * QB, :].T)
        in_maps.append({
            "xq": xq_t, "xk": xk_t[b], "xv": xv_t[b],
            "wq": wq_f, "wk": wk_f, "wv": wv_f, "wo": wo_f,
            "bq": bq_f, "bk": bk_f, "bo": bo_eff,
        })
    return in_maps


def assemble(results):
    outp = np.empty((B, S, E), dtype=np.float32)
    for core in range(NCORES):
        b = core // (NCORES // B)
        qc = core % (NCORES // B)
        outp[b, qc * QB:(qc + 1) * QB, :] = results[core]["out"]
    return outp


def kernel(query, key_, value, Wq, bq, Wk, bk, Wv, bv, Wo, bo):
    nc = _get_nc()
    in_maps = make_in_maps(query, key_, value, Wq, bq, Wk, bk, Wv, bv, Wo, bo)
    res = run_bass_kernel_spmd(nc, in_maps, core_ids=list(range(NCORES)))
    return assemble(res.results)


# revision 11
# speedup vs baseline: 2.3785x; 2.3785x over previous
# Multi-head attention kernel for Trainium2, sharded over 8 NeuronCores.
#
# Sharding: core = (batch b, query-chunk qc). Each core handles QB=512 queries
# of one batch, all 12 heads, recomputing the K/V projections for its batch
# (cheaper than cross-core collectives on this chip).
#
# Layout strategy (all fp32):
#   - Host pre-transposes activations to [E, S] so the contraction dim (E)
#     lands on SBUF partitions.
#   - q^T, k^T computed as [768, S] via lhsT=W chunks; per-partition bias
#     added during PSUM->SBUF copy.
#   - v computed directly as [keys, 768] using x_v^T chunks as the stationary
#     operand; stored with a ones-column per head ([128,16,12,65]) so the PV
#     matmul (M=65) also produces the softmax denominator row.
#   - scores^T = [keys, queries] per head: K=64 matmuls; even/odd heads sit in
#     partition halves 0-63/64-127 so adjacent matmuls land in disjoint PE row
#     groups and can run concurrently.
#   - exp on ScalarE in [128, 2*512] groups, PSUM->SBUF, streamed straight
#     into the accumulating PV matmul (no full score matrix ever lives in SBUF).
#   - normalize: reciprocal of denom row + gpsimd partition_broadcast + DVE mul.
#   - output projection contracts d_all via K=64 chunks (head-pair row packed),
#     epilogue adds host-precomputed bias (bv folded through Wo).

import numpy as np
from contextlib import ExitStack

import concourse.bass as bass
import concourse.mybir as mybir
import concourse.tile as tile
from concourse import bacc
from concourse.bass_utils import run_bass_kernel_spmd

F32 = mybir.dt.float32
BF16 = mybir.dt.bfloat16
P = 128
E = 768
S = 2048
B = 2
H = 12
D = 64
QB = 512          # queries per core
NCORES = 8
EC = E // P       # 6 e-chunks
KT = S // P       # 16 key tiles
MT_Q = E // P     # 6 M-tiles for q^T/k^T (768 rows)
NC4 = S // 512    # 4 n-slices of k^T


def build_nc():
    nc = bacc.Bacc("TRN2", debug=False)

    # DRAM I/O (per-core shapes; same NEFF on all 8 cores)
    xq = nc.dram_tensor("xq", (E, QB), BF16, kind="ExternalInput")     # query[b,chunk].T
    xk = nc.dram_tensor("xk", (E, S), BF16, kind="ExternalInput")      # key[b].T
    xv = nc.dram_tensor("xv", (E, S), BF16, kind="ExternalInput")      # value[b].T
    wq = nc.dram_tensor("wq", (E, E), BF16, kind="ExternalInput")      # [E, H*D], pre-scaled 1/sqrt(D)
    wk = nc.dram_tensor("wk", (E, E), BF16, kind="ExternalInput")
    wv = nc.dram_tensor("wv", (E, E), BF16, kind="ExternalInput")
    wo = nc.dram_tensor("wo", (E, E), BF16, kind="ExternalInput")
    bq = nc.dram_tensor("bq", (P, MT_Q), F32, kind="ExternalInput")   # per-partition bias per M-tile
    bk = nc.dram_tensor("bk", (P, MT_Q), F32, kind="ExternalInput")
    bo = nc.dram_tensor("bo", (P, E), F32, kind="ExternalInput")      # bv@Wo + bo, broadcast
    out = nc.dram_tensor("out", (QB, E), F32, kind="ExternalOutput")

    with tile.TileContext(nc) as tc:
        with ExitStack() as ctx:
            _emit(ctx, tc, nc, xq, xk, xv, wq, wk, wv, wo, bq, bk, bo, out)
    nc.compile()
    return nc


def _emit(ctx, tc, nc, xq, xk, xv, wq, wk, wv, wo, bq, bk, bo, out):
    # ---- pools ----
    # SBUF persistent
    persist = ctx.enter_context(tc.tile_pool(name="persist", bufs=1))
    # big weight slots [128, 6, 768] reused wq -> wk -> wv -> wo
    wpool = ctx.enter_context(tc.tile_pool(name="wpool", bufs=2))
    # x input slices
    xpool = ctx.enter_context(tc.tile_pool(name="xpool", bufs=2))
    xvpool = ctx.enter_context(tc.tile_pool(name="xvpool", bufs=2))
    # exp output stream
    epool = ctx.enter_context(tc.tile_pool(name="epool", bufs=2))
    # small temps
    spool = ctx.enter_context(tc.tile_pool(name="spool", bufs=1))
    outpool = ctx.enter_context(tc.tile_pool(name="outpool", bufs=1))
    # PSUM pools
    psA = ctx.enter_context(tc.tile_pool(name="psA", bufs=2, space="PSUM"))   # [128,512] proj qk + PV out
    psB = ctx.enter_context(tc.tile_pool(name="psB", bufs=1, space="PSUM"))   # [128,768] v proj + out proj
    psC = ctx.enter_context(tc.tile_pool(name="psC", bufs=2, space="PSUM"))   # [128,2,512] scores

    # ---- persistent SBUF tensors ----
    qT = persist.tile([P, MT_Q, QB], BF16)       # q^T [768, QB]
    kT = persist.tile([P, MT_Q, S], BF16)        # k^T [768, S]
    v_sb = persist.tile([P, KT, H, D + 1], BF16)  # v + ones column per head
    o_all = persist.tile([P, H // 2, QB], BF16)   # normalized o^T, head pairs in partition halves
    bq_sb = persist.tile([P, MT_Q], F32)
    bk_sb = persist.tile([P, MT_Q], F32)
    bo_sb = persist.tile([P, E], F32)
    ones_sb = persist.tile([D + 1, D], F32)  # row D holds ones, base partition D

    nc.sync.dma_start(bq_sb[:], bq[:])
    nc.sync.dma_start(bk_sb[:], bk[:])
    nc.sync.dma_start(bo_sb[:], bo[:])

    # ones columns for denominator (written once; v-proj copies don't touch col D)
    nc.vector.memset(v_sb[:, :, :, D], 1.0)
    nc.vector.memset(ones_sb[D:D + 1, :], 1.0)

    wq_t = wpool.tile([P, EC, E], BF16, tag="w18")
    nc.sync.dma_start(wq_t[:], wq[:].rearrange("(ec p) m -> p ec m", p=P))

    # ---- q^T projection ----
    xq_t = xpool.tile([P, EC, QB], BF16, tag="xs")
    nc.sync.dma_start(xq_t[:], xq[:].rearrange("(ec p) s -> p ec s", p=P))
    for mt in range(MT_Q):
        ps = psA.tile([P, 512], F32, tag="psA")
        for ec in range(EC):
            nc.tensor.matmul(ps[:], wq_t[:, ec, mt * P:(mt + 1) * P], xq_t[:, ec, :],
                             start=(ec == 0), stop=(ec == EC - 1))
        nc.vector.tensor_scalar_add(qT[:, mt, :], ps[:], bq_sb[:, mt:mt + 1])

    # ---- k^T projection ----
    wk_t = wpool.tile([P, EC, E], BF16, tag="w18")
    nc.sync.dma_start(wk_t[:], wk[:].rearrange("(ec p) m -> p ec m", p=P))
    for n4 in range(NC4):
        xk_t = xpool.tile([P, EC, 512], BF16, tag="xs")
        nc.sync.dma_start(xk_t[:], xk[:, n4 * 512:(n4 + 1) * 512].rearrange("(ec p) s -> p ec s", p=P))
        for mt in range(MT_Q):
            ps = psA.tile([P, 512], F32, tag="psA")
            for ec in range(EC):
                nc.tensor.matmul(ps[:], wk_t[:, ec, mt * P:(mt + 1) * P], xk_t[:, ec, :],
                                 start=(ec == 0), stop=(ec == EC - 1))
            nc.vector.tensor_scalar_add(kT[:, mt, n4 * 512:(n4 + 1) * 512], ps[:], bk_sb[:, mt:mt + 1])

    # ---- v projection (direct [keys, d]; no bias — folded into bo host-side) ----
    wv_t = wpool.tile([P, EC, E], BF16, tag="w18")
    nc.sync.dma_start(wv_t[:], wv[:].rearrange("(ec p) m -> p ec m", p=P))
    for kt in range(KT):
        xv_t = xvpool.tile([P, EC, P], BF16, tag="xv")
        nc.sync.dma_start(xv_t[:], xv[:, kt * P:(kt + 1) * P].rearrange("(ec p) s -> p ec s", p=P))
        psv = psB.tile([P, E], F32, tag="psB")
        for ec in range(EC):
            nc.tensor.matmul(psv[:, 0:512], xv_t[:, ec, :], wv_t[:, ec, 0:512],
                             start=(ec == 0), stop=(ec == EC - 1))
            nc.tensor.matmul(psv[:, 512:768], xv_t[:, ec, :], wv_t[:, ec, 512:768],
                             start=(ec == 0), stop=(ec == EC - 1))
        # strided copy into per-head slots (leaves ones column intact)
        nc.vector.tensor_copy(v_sb[:, kt, :, 0:D], psv[:].rearrange("p (h d) -> p h d", d=D))

    # ---- attention: head pairs ----
    NG = KT // 2  # 8 groups of 2 key tiles
    for hp in range(H // 2):
        o_ps = {}
        for i in range(2):
            o_ps[i] = psA.tile([P, 512], F32, tag="psA", name=f"o_ps{i}")
        for g in range(NG):
            for i in range(2):
                h = 2 * hp + i
                po = D * i      # partition offset of this head's d-rows
                mt = h // 2
                st = psC.tile([P, 2, 512], F32, tag="psC")
                for j in range(2):
                    kt = 2 * g + j
                    nc.tensor.matmul(st[:, j, :],
                                     kT[po:po + D, mt, kt * P:(kt + 1) * P],
                                     qT[po:po + D, mt, :],
                                     start=True, stop=True)
                ex = epool.tile([P, 2, 512], BF16, tag="ex")
                nc.scalar.activation(ex[:, :, :], st[:, :, :], mybir.ActivationFunctionType.Exp)
                for j in range(2):
                    kt = 2 * g + j
                    nc.tensor.matmul(o_ps[i][0:D + 1, :],
                                     v_sb[:, kt, h, :],
                                     ex[:, j, :],
                                     start=(g == 0 and j == 0), stop=(g == NG - 1 and j == 1))
        # normalize: recip of denom row, PE-broadcast to 64 partitions, multiply
        for i in range(2):
            po = D * i
            rec = spool.tile([D + 1, 512], F32, tag="rec", name=f"rec{i}")
            nc.vector.reciprocal(rec[D:D + 1, :], o_ps[i][D:D + 1, :])
            bc_ps = psB.tile([P, E], F32, tag="psB", name=f"bc{i}")
            nc.tensor.matmul(bc_ps[0:D, 0:512], ones_sb[D:D + 1, :], rec[D:D + 1, :],
                             start=True, stop=True)
            bc_sb = spool.tile([D, 512], F32, tag="rb", name=f"bc_sb{i}")
            nc.scalar.copy(bc_sb[:], bc_ps[0:D, 0:512])
            nc.vector.tensor_tensor(o_all[po:po + D, hp, :], o_ps[i][0:D, :], bc_sb[:],
                                    mybir.AluOpType.mult)

    # ---- output projection ----
    wo_t = wpool.tile([P, EC, E], BF16, tag="w18")
    nc.sync.dma_start(wo_t[:], wo[:].rearrange("(ec p) m -> p ec m", p=P))
    ST = QB // P  # 4 s-tiles
    for st4 in range(ST):
        op = psB.tile([P, E], F32, tag="psB")
        for hp in range(H // 2):
            # both heads of the pair contract in one K=128 matmul
            first = (hp == 0)
            last = (hp == H // 2 - 1)
            nc.tensor.matmul(op[:, 0:512],
                             o_all[:, hp, st4 * P:(st4 + 1) * P],
                             wo_t[:, hp, 0:512],
                             start=first, stop=last)
            nc.tensor.matmul(op[:, 512:768],
                             o_all[:, hp, st4 * P:(st4 + 1) * P],
                             wo_t[:, hp, 512:768],
                             start=first, stop=last)
        out_sb = outpool.tile([P, E], F32, tag="outsb")
        nc.vector.tensor_tensor(out_sb[:], op[:], bo_sb[:], mybir.AluOpType.add)
        nc.sync.dma_start(out[st4 * P:(st4 + 1) * P, :], out_sb[:])


_NC_CACHE = None


def _get_nc():
    global _NC_CACHE
    if _NC_CACHE is None:
        _NC_CACHE = build_nc()
    return _NC_CACHE


def make_in_maps(query, key_, value, Wq, bq, Wk, bk, Wv, bv, Wo, bo):
    """Host-side sharding + layout prep. Returns list of 8 input dicts."""
    query = np.asarray(query, dtype=np.float32)
    key_ = np.asarray(key_, dtype=np.float32)
    value = np.asarray(value, dtype=np.float32)
    scale = 1.0 / np.sqrt(np.float32(D))

    import ml_dtypes
    BF = ml_dtypes.bfloat16
    wq_f = (np.ascontiguousarray(np.transpose(np.asarray(Wq, np.float32), (1, 0, 2)).reshape(E, E)) * scale).astype(BF)
    wk_f = np.ascontiguousarray(np.transpose(np.asarray(Wk, np.float32), (1, 0, 2)).reshape(E, E)).astype(BF)
    wv_f = np.ascontiguousarray(np.transpose(np.asarray(Wv, np.float32), (1, 0, 2)).reshape(E, E)).astype(BF)
    wo_f = np.ascontiguousarray(np.asarray(Wo, np.float32)).astype(BF)

    bq_f = (np.asarray(bq, np.float32).reshape(E) * scale).reshape(MT_Q, P).T.copy()
    bk_f = np.asarray(bk, np.float32).reshape(E).reshape(MT_Q, P).T.copy()
    bv_f = np.asarray(bv, np.float32).reshape(E)
    wo_f32 = wo_f.astype(np.float32)
    bo_eff = np.tile((bv_f @ wo_f32 + np.asarray(bo, np.float32)).reshape(1, E), (P, 1)).copy()

    xk_t = [np.ascontiguousarray(key_[b].T).astype(BF) for b in range(B)]
    xv_t = [np.ascontiguousarray(value[b].T).astype(BF) for b in range(B)]

    in_maps = []
    for core in range(NCORES):
        b = core // (NCORES // B)
        qc = core % (NCORES // B)
        xq_t = np.ascontiguousarray(query[b, qc * QB:(qc + 1) * QB, :].T).astype(BF)
        in_maps.append({
            "xq": xq_t, "xk": xk_t[b], "xv": xv_t[b],
            "wq": wq_f, "wk": wk_f, "wv": wv_f, "wo": wo_f,
            "bq": bq_f, "bk": bk_f, "bo": bo_eff,
        })
    return in_maps


def assemble(results):
    outp = np.empty((B, S, E), dtype=np.float32)
    for core in range(NCORES):
        b = core // (NCORES // B)
        qc = core % (NCORES // B)
        outp[b, qc * QB:(qc + 1) * QB, :] = results[core]["out"]
    return outp


def kernel(query, key_, value, Wq, bq, Wk, bk, Wv, bv, Wo, bo):
    nc = _get_nc()
    in_maps = make_in_maps(query, key_, value, Wq, bq, Wk, bk, Wv, bv, Wo, bo)
    res = run_bass_kernel_spmd(nc, in_maps, core_ids=list(range(NCORES)))
    return assemble(res.results)


# revision 18
# speedup vs baseline: 2.8400x; 1.1940x over previous
# Multi-head attention kernel for Trainium2, sharded over 8 NeuronCores.
#
# Sharding: core = (batch b, query-chunk qc). Each core handles QB=512 queries
# of one batch, all 12 heads, recomputing the K/V projections for its batch
# (cheaper than cross-core collectives on this chip).
#
# Layout strategy (all fp32):
#   - Host pre-transposes activations to [E, S] so the contraction dim (E)
#     lands on SBUF partitions.
#   - q^T, k^T computed as [768, S] via lhsT=W chunks; per-partition bias
#     added during PSUM->SBUF copy.
#   - v computed directly as [keys, 768] using x_v^T chunks as the stationary
#     operand; stored with a ones-column per head ([128,16,12,65]) so the PV
#     matmul (M=65) also produces the softmax denominator row.
#   - scores^T = [keys, queries] per head: K=64 matmuls; even/odd heads sit in
#     partition halves 0-63/64-127 so adjacent matmuls land in disjoint PE row
#     groups and can run concurrently.
#   - exp on ScalarE in [128, 2*512] groups, PSUM->SBUF, streamed straight
#     into the accumulating PV matmul (no full score matrix ever lives in SBUF).
#   - normalize: reciprocal of denom row + gpsimd partition_broadcast + DVE mul.
#   - output projection contracts d_all via K=64 chunks (head-pair row packed),
#     epilogue adds host-precomputed bias (bv folded through Wo).

import numpy as np
from contextlib import ExitStack

import concourse.bass as bass
import concourse.mybir as mybir
import concourse.tile as tile
from concourse import bacc
from concourse.bass_utils import run_bass_kernel_spmd

F32 = mybir.dt.float32
BF16 = mybir.dt.bfloat16
P = 128
E = 768
S = 2048
B = 2
H = 12
D = 64
QB = 512          # queries per core
NCORES = 8
EC = E // P       # 6 e-chunks
KT = S // P       # 16 key tiles
MT_Q = E // P     # 6 M-tiles for q^T/k^T (768 rows)
NC4 = S // 512    # 4 n-slices of k^T


def build_nc():
    nc = bacc.Bacc("TRN2", debug=False)

    # DRAM I/O (per-core shapes; same NEFF on all 8 cores)
    xq = nc.dram_tensor("xq", (E, QB), BF16, kind="ExternalInput")     # query[b,chunk].T
    xk = nc.dram_tensor("xk", (E, S), BF16, kind="ExternalInput")      # key[b].T
    xv = nc.dram_tensor("xv", (E, S), BF16, kind="ExternalInput")      # value[b].T
    wq = nc.dram_tensor("wq", (E, E), BF16, kind="ExternalInput")      # [E, H*D], pre-scaled 1/sqrt(D)
    wk = nc.dram_tensor("wk", (E, E), BF16, kind="ExternalInput")
    wv = nc.dram_tensor("wv", (E, E), BF16, kind="ExternalInput")
    wo = nc.dram_tensor("wo", (E, E), BF16, kind="ExternalInput")
    bq = nc.dram_tensor("bq", (P, MT_Q), F32, kind="ExternalInput")   # per-partition bias per M-tile
    bk = nc.dram_tensor("bk", (P, MT_Q), F32, kind="ExternalInput")
    bo = nc.dram_tensor("bo", (P, E), F32, kind="ExternalInput")      # bv@Wo + bo, broadcast
    seld = nc.dram_tensor("seld", (H, H * D), F32, kind="ExternalInput")  # head-broadcast selector
    out = nc.dram_tensor("out", (QB, E), F32, kind="ExternalOutput")

    with tile.TileContext(nc) as tc:
        with ExitStack() as ctx:
            _emit(ctx, tc, nc, xq, xk, xv, wq, wk, wv, wo, bq, bk, bo, seld, out)
    nc.compile()
    return nc


def _emit(ctx, tc, nc, xq, xk, xv, wq, wk, wv, wo, bq, bk, bo, seld, out):
    # ---- pools ----
    # SBUF persistent
    persist = ctx.enter_context(tc.tile_pool(name="persist", bufs=1))
    # big weight slots [128, 6, 768] reused wq -> wk -> wv -> wo
    wpool = ctx.enter_context(tc.tile_pool(name="wpool", bufs=2))
    # x input slices
    xpool = ctx.enter_context(tc.tile_pool(name="xpool", bufs=2))
    xvpool = ctx.enter_context(tc.tile_pool(name="xvpool", bufs=3))
    # exp output stream
    epool = ctx.enter_context(tc.tile_pool(name="epool", bufs=4))
    # small temps
    spool = ctx.enter_context(tc.tile_pool(name="spool", bufs=2))
    outpool = ctx.enter_context(tc.tile_pool(name="outpool", bufs=2))
    # PSUM pools
    psA = ctx.enter_context(tc.tile_pool(name="psA", bufs=2, space="PSUM"))   # [128,512] proj qk + PV out
    psB = ctx.enter_context(tc.tile_pool(name="psB", bufs=1, space="PSUM"))   # [128,768] v proj + out proj
    psC = ctx.enter_context(tc.tile_pool(name="psC", bufs=2, space="PSUM"))   # [128,2,512] scores

    # ---- persistent SBUF tensors ----
    qT = persist.tile([P, MT_Q, QB], BF16)       # q^T [768, QB]
    kT = persist.tile([P, MT_Q, S], BF16)        # k^T [768, S]
    v_sb = persist.tile([P, KT, H, D + 1], BF16)  # v + ones column per head
    o_all = persist.tile([P, H // 2, QB], BF16)   # normalized o^T, head pairs in partition halves
    bq_sb = persist.tile([P, MT_Q], F32)
    bk_sb = persist.tile([P, MT_Q], F32)
    bo_sb = persist.tile([P, E], F32)
    o_raw = persist.tile([D + 1, H, 512], F32)   # unnormalized o^T + denom row per head
    dens = persist.tile([H, 512], F32)           # gathered denominators
    drec = persist.tile([H, 512], F32)           # their reciprocals
    sel = persist.tile([H, H * D], F32)          # selector: sel[h, h*D:(h+1)*D] = 1

    nc.sync.dma_start(bq_sb[:], bq[:])
    nc.sync.dma_start(bk_sb[:], bk[:])
    nc.sync.dma_start(bo_sb[:], bo[:])

    # ones columns for denominator (written once; v-proj copies don't touch col D)
    nc.vector.memset(v_sb[:, :, :, D], 1.0)
    nc.sync.dma_start(sel[:], seld[:])

    wq_t = wpool.tile([P, EC, E], BF16, tag="w18")
    nc.sync.dma_start(wq_t[:], wq[:].rearrange("(ec p) m -> p ec m", p=P))

    # ---- q^T projection ----
    xq_t = xpool.tile([P, EC, QB], BF16, tag="xs")
    nc.sync.dma_start(xq_t[:], xq[:].rearrange("(ec p) s -> p ec s", p=P))
    for mt in range(MT_Q):
        ps = psA.tile([P, 512], F32, tag="psA")
        for ec in range(EC):
            nc.tensor.matmul(ps[:], wq_t[:, ec, mt * P:(mt + 1) * P], xq_t[:, ec, :],
                             start=(ec == 0), stop=(ec == EC - 1))
        nc.vector.tensor_scalar_add(qT[:, mt, :], ps[:], bq_sb[:, mt:mt + 1])

    # ---- k^T projection ----
    wk_t = wpool.tile([P, EC, E], BF16, tag="w18")
    nc.sync.dma_start(wk_t[:], wk[:].rearrange("(ec p) m -> p ec m", p=P))
    for n4 in range(NC4):
        xk_t = xpool.tile([P, EC, 512], BF16, tag="xs")
        nc.sync.dma_start(xk_t[:], xk[:, n4 * 512:(n4 + 1) * 512].rearrange("(ec p) s -> p ec s", p=P))
        for mt in range(MT_Q):
            ps = psA.tile([P, 512], F32, tag="psA")
            for ec in range(EC):
                nc.tensor.matmul(ps[:], wk_t[:, ec, mt * P:(mt + 1) * P], xk_t[:, ec, :],
                                 start=(ec == 0), stop=(ec == EC - 1))
            nc.vector.tensor_scalar_add(kT[:, mt, n4 * 512:(n4 + 1) * 512], ps[:], bk_sb[:, mt:mt + 1])

    # ---- v projection (direct [keys, d]; no bias — folded into bo host-side) ----
    wv_t = wpool.tile([P, EC, E], BF16, tag="w18")
    nc.sync.dma_start(wv_t[:], wv[:].rearrange("(ec p) m -> p ec m", p=P))
    for kt in range(KT):
        xv_t = xvpool.tile([P, EC, P], BF16, tag="xv")
        nc.sync.dma_start(xv_t[:], xv[:, kt * P:(kt + 1) * P].rearrange("(ec p) s -> p ec s", p=P))
        psv = psB.tile([P, E], F32, tag="psB")
        for ec in range(EC):
            nc.tensor.matmul(psv[:, 0:512], xv_t[:, ec, :], wv_t[:, ec, 0:512],
                             start=(ec == 0), stop=(ec == EC - 1))
            nc.tensor.matmul(psv[:, 512:768], xv_t[:, ec, :], wv_t[:, ec, 512:768],
                             start=(ec == 0), stop=(ec == EC - 1))
        # strided copy into per-head slots (leaves ones column intact)
        nc.vector.tensor_copy(v_sb[:, kt, :, 0:D], psv[:].rearrange("p (h d) -> p h d", d=D))

    # ---- attention: head pairs ----
    # Per key tile: both heads' score matmuls are adjacent K=64 ops on
    # disjoint PE row groups (partitions 0-63 / 64-127) -> run concurrently.
    for hp in range(H // 2):
        o_ps = {}
        for i in range(2):
            o_ps[i] = psA.tile([P, 512], F32, tag="psA", name=f"o_ps{i}")
        for kt in range(KT):
            st = psC.tile([P, 2, 512], F32, tag="psC")
            for i in range(2):
                po = D * i      # partition offset of this head's d-rows
                nc.tensor.matmul(st[:, i, :],
                                 kT[po:po + D, hp, kt * P:(kt + 1) * P],
                                 qT[po:po + D, hp, :],
                                 start=True, stop=True)
            ex = epool.tile([P, 2, 512], BF16, tag="ex")
            nc.scalar.activation(ex[:, :, :], st[:, :, :], mybir.ActivationFunctionType.Exp)
            for i in range(2):
                nc.tensor.matmul(o_ps[i][0:D + 1, :],
                                 v_sb[:, kt, 2 * hp + i, :],
                                 ex[:, i, :],
                                 start=(kt == 0), stop=(kt == KT - 1))
        # stage unnormalized outputs (fast PSUM release)
        for i in range(2):
            nc.vector.tensor_copy(o_raw[:, 2 * hp + i, :], o_ps[i][0:D + 1, :])

    # ---- batched softmax normalization ----
    # gather the 12 denominator rows (partition D, one per head) onto
    # partitions 0..11 (DRAM bounce reshapes), then a single batched reciprocal.
    dram = ctx.enter_context(tc.tile_pool(name="dram", bufs=1, space="DRAM"))
    dtmp = dram.tile([1, H * 512], F32)
    nc.sync.dma_start(dtmp[:], o_raw[D:D + 1, :, :])
    nc.sync.dma_start(dens[:], dtmp[0, :].rearrange("(h q) -> h q", q=512))
    nc.vector.reciprocal(drec[:], dens[:])
    for hp in range(H // 2):
        for i in range(2):
            h = 2 * hp + i
            po = D * i
            bc_ps = psB.tile([P, E], F32, tag="psB", name=f"bc{i}")
            nc.tensor.matmul(bc_ps[0:D, 0:512], sel[:, h * D:(h + 1) * D], drec[:],
                             start=True, stop=True)
            bc_sb = spool.tile([D, 512], F32, tag="rb", name=f"bc_sb{i}")
            nc.scalar.copy(bc_sb[:], bc_ps[0:D, 0:512])
            nc.vector.tensor_tensor(o_all[po:po + D, hp, :], o_raw[0:D, h, :], bc_sb[:],
                                    mybir.AluOpType.mult)

    # ---- output projection ----
    wo_t = wpool.tile([P, EC, E], BF16, tag="w18")
    nc.sync.dma_start(wo_t[:], wo[:].rearrange("(ec p) m -> p ec m", p=P))
    ST = QB // P  # 4 s-tiles
    for st4 in range(ST):
        op = psB.tile([P, E], F32, tag="psB")
        for hp in range(H // 2):
            # both heads of the pair contract in one K=128 matmul
            first = (hp == 0)
            last = (hp == H // 2 - 1)
            nc.tensor.matmul(op[:, 0:512],
                             o_all[:, hp, st4 * P:(st4 + 1) * P],
                             wo_t[:, hp, 0:512],
                             start=first, stop=last)
            nc.tensor.matmul(op[:, 512:768],
                             o_all[:, hp, st4 * P:(st4 + 1) * P],
                             wo_t[:, hp, 512:768],
                             start=first, stop=last)
        out_sb = outpool.tile([P, E], F32, tag="outsb")
        nc.vector.tensor_tensor(out_sb[:], op[:], bo_sb[:], mybir.AluOpType.add)
        nc.sync.dma_start(out[st4 * P:(st4 + 1) * P, :], out_sb[:])


_NC_CACHE = None


def _get_nc():
    global _NC_CACHE
    if _NC_CACHE is None:
        _NC_CACHE = build_nc()
    return _NC_CACHE


def make_in_maps(query, key_, value, Wq, bq, Wk, bk, Wv, bv, Wo, bo):
    """Host-side sharding + layout prep. Returns list of 8 input dicts."""
    query = np.asarray(query, dtype=np.float32)
    key_ = np.asarray(key_, dtype=np.float32)
    value = np.asarray(value, dtype=np.float32)
    scale = 1.0 / np.sqrt(np.float32(D))

    import ml_dtypes
    BF = ml_dtypes.bfloat16
    wq_f = (np.ascontiguousarray(np.transpose(np.asarray(Wq, np.float32), (1, 0, 2)).reshape(E, E)) * scale).astype(BF)
    wk_f = np.ascontiguousarray(np.transpose(np.asarray(Wk, np.float32), (1, 0, 2)).reshape(E, E)).astype(BF)
    wv_f = np.ascontiguousarray(np.transpose(np.asarray(Wv, np.float32), (1, 0, 2)).reshape(E, E)).astype(BF)
    wo_f = np.ascontiguousarray(np.asarray(Wo, np.float32)).astype(BF)

    bq_f = (np.asarray(bq, np.float32).reshape(E) * scale).reshape(MT_Q, P).T.copy()
    bk_f = np.asarray(bk, np.float32).reshape(E).reshape(MT_Q, P).T.copy()
    bv_f = np.asarray(bv, np.float32).reshape(E)
    wo_f32 = wo_f.astype(np.float32)
    bo_eff = np.tile((bv_f @ wo_f32 + np.asarray(bo, np.float32)).reshape(1, E), (P, 1)).copy()

    xk_t = [np.ascontiguousarray(key_[b].T).astype(BF) for b in range(B)]
    xv_t = [np.ascontiguousarray(value[b].T).astype(BF) for b in range(B)]

    sel_np = np.zeros((H, H * D), dtype=np.float32)
    for h in range(H):
        sel_np[h, h * D:(h + 1) * D] = 1.0

    in_maps = []
    for core in range(NCORES):
        b = core // (NCORES // B)
        qc = core % (NCORES // B)
        xq_t = np.ascontiguousarray(query[b, qc * QB:(qc + 1) * QB, :].T).astype(BF)
        in_maps.append({
            "xq": xq_t, "xk": xk_t[b], "xv": xv_t[b],
            "wq": wq_f, "wk": wk_f, "wv": wv_f, "wo": wo_f,
            "bq": bq_f, "bk": bk_f, "bo": bo_eff, "seld": sel_np,
        })
    return in_maps


def assemble(results):
    outp = np.empty((B, S, E), dtype=np.float32)
    for core in range(NCORES):
        b = core // (NCORES // B)
        qc = core % (NCORES // B)
        outp[b, qc * QB:(qc + 1) * QB, :] = results[core]["out"]
    return outp


def kernel(query, key_, value, Wq, bq, Wk, bk, Wv, bv, Wo, bo):
    nc = _get_nc()
    in_maps = make_in_maps(query, key_, value, Wq, bq, Wk, bk, Wv, bv, Wo, bo)
    res = run_bass_kernel_spmd(nc, in_maps, core_ids=list(range(NCORES)))
    return assemble(res.results)
